# revision 10
# baseline (speedup 1.0000x reference)
"""Trainium2 Bass kernel for nn_Decoder (seq2seq LSTM decoder w/ attention).

Strategy (8 NeuronCores, SPMD, zero collectives):
  - The LSTM recurrence, embedding gathers and attention are replicated on
    every core (the recurrence is weight-stream bound on the PE, so running
    the full batch B=32 costs the same as any shard of it; per-step
    collectives would cost ~20us each and are a non-starter).
  - The 32000-wide output projection is sharded over vocab: each core owns a
    4096-row slice of W_out (32000 padded to 32768 = 8*4096) and produces
    logits[:, shard]. The host concatenates shards and drops the padding.

Per-core pipeline:
  A: weight transposes (PE), encoder-embedding gathers, avg/h0, and the
     time-parallel LSTM input projection Zx = X @ W_ih.T + b (to DRAM).
  B: 128 sequential LSTM steps; z = Zx[t] + h @ W_hh.T via PE matmuls with
     the hidden state transposed into lhsT form each step by PE transposes.
  C: per-sentence attention (masked softmax via an additive -30 mask row
     folded into the energy matmul accumulation) producing combined.T
     blocks in DRAM.
  D: the projection combined @ W_out_shard.T + b_out in 4 vocab quarters,
     keeping each transposed weight quarter resident in SBUF.

All large matmuls run as float32r (full PE rate); data stays fp32 end to end.
"""

import numpy as np

import concourse.bass as bass
import concourse.tile as tile
from concourse import bacc
from concourse import mybir
from concourse.bass_utils import run_bass_kernel_spmd
from concourse.masks import make_identity

B, TD, TE = 32, 128, 128
E, H = 512, 1024
E2 = 2 * E                 # 1024
G = 4 * H                  # 4096 (gates)
KE, KH, K2E = E // 128, H // 128, E2 // 128   # 4, 8, 8
V = 32000
VPAD = 32768               # 8 * 4096
VS = VPAD // 8             # 4096 per-core vocab shard (padded)
NQ = 4                     # vocab quarters in phase D
QW = VS // NQ              # 1024 cols per quarter
NCORES = 8

F32 = mybir.dt.float32
F32R = mybir.dt.float32r
I32 = mybir.dt.int32
AF = mybir.ActivationFunctionType
OP = mybir.AluOpType


def r(ap):
    """View an fp32 AP as float32r for full-rate PE matmuls."""
    return ap.bitcast(F32R)


def build_nc():
    nc = bacc.Bacc()

    tgt = nc.dram_tensor("tgt", [B, TD], I32, kind="ExternalInput").ap()
    srcs = nc.dram_tensor("srcs", [B, TE], I32, kind="ExternalInput").ap()
    slen = nc.dram_tensor("slen", [B], I32, kind="ExternalInput").ap()
    arange = nc.dram_tensor("arange", [TE], I32, kind="ExternalInput").ap()
    ence = nc.dram_tensor("ence", [V, E], F32, kind="ExternalInput").ap()
    dece = nc.dram_tensor("dece", [V, E], F32, kind="ExternalInput").ap()
    pose = nc.dram_tensor("pose", [TE, E], F32, kind="ExternalInput").ap()
    wih = nc.dram_tensor("wih", [G, E], F32, kind="ExternalInput").ap()
    whh = nc.dram_tensor("whh", [G, H], F32, kind="ExternalInput").ap()
    bihh = nc.dram_tensor("bihh", [G], F32, kind="ExternalInput").ap()
    wh0 = nc.dram_tensor("wh0", [H, E2], F32, kind="ExternalInput").ap()
    bh0 = nc.dram_tensor("bh0", [H], F32, kind="ExternalInput").ap()
    wout = nc.dram_tensor("wout", [VS, E2 + H], F32, kind="ExternalInput").ap()
    bout = nc.dram_tensor("bout", [VS], F32, kind="ExternalInput").ap()
    out = nc.dram_tensor("out", [B * TD, VS], F32, kind="ExternalOutput").ap()

    with tile.TileContext(nc) as tc:
        with (
            tc.tile_pool(name="dram", bufs=1, space="DRAM") as dp,
            tc.tile_pool(name="const", bufs=1) as cp,
        ):
            zx_d = dp.tile([TD, B, G], F32)          # LSTM input proj, (t, b, 4H)
            lstm_d = dp.tile([B, TD, H], F32)        # h_t rows
            combT_d = dp.tile([B, 16, 128, TD], F32) # combined.T blocks per sentence

            # ---------------- constants (whole-kernel residents) ----------------
            ident = cp.tile([128, 128], F32)
            make_identity(nc, ident)
            ones_col = cp.tile([128, 1], F32)        # 1/TE for the encoder mean
            nc.vector.memset(ones_col, 1.0 / TE)
            ones_row = cp.tile([1, 128], F32)        # lhsT for mask broadcast matmul
            nc.vector.memset(ones_row, 1.0)

            posemb = cp.tile([TE, E], F32)
            nc.sync.dma_start(out=posemb, in_=pose)
            bh0_sb = cp.tile([128, KH], F32)
            nc.sync.dma_start(out=bh0_sb, in_=bh0.rearrange("(k p) -> p k", p=128))
            srcidx = cp.tile([TE, B], I32)
            nc.sync.dma_start(out=srcidx, in_=srcs.rearrange("b t -> t b"))
            tgtidx = cp.tile([TD, B], I32)
            nc.sync.dma_start(out=tgtidx, in_=tgt.rearrange("b t -> t b"))
            iota_i = cp.tile([1, TE], I32)
            nc.sync.dma_start(out=iota_i, in_=arange[None, :])
            iota_f = cp.tile([1, TE], F32)
            nc.vector.tensor_copy(out=iota_f, in_=iota_i)
            slen_i = cp.tile([1, B], I32)
            nc.sync.dma_start(out=slen_i, in_=slen[None, :])
            slen_f = cp.tile([1, B], F32)
            nc.vector.tensor_copy(out=slen_f, in_=slen_i)

            posembT = cp.tile([128, KE * 128], F32)  # (e within chunk, ec*128 + t)
            h0T = cp.tile([128, KH * 32], F32)       # (h within chunk, k*32 + b)
            h0 = cp.tile([B, H], F32)

            # ================= phase A =================
            with (
                tc.tile_pool(name="wA", bufs=1) as wa,
                tc.tile_pool(name="psA", bufs=1, space="PSUM") as psa,
                tc.tile_pool(name="sbA", bufs=1) as sba,
            ):
                wihT = wa.tile([128, KE, G], F32)    # (e, ec, gate)
                wh0T = wa.tile([128, K2E, H], F32)   # (e2, kc, h)
                bias_g = sba.tile([1, G], F32)
                nc.sync.dma_start(out=bias_g, in_=bihh[None, :])

                # --- W_ih.T ---
                for gi in range(G // 128):
                    wt = sba.tile([128, E], F32, tag="wld", bufs=3)
                    nc.sync.dma_start(out=wt, in_=wih[gi * 128 : (gi + 1) * 128, :])
                    for ek in range(KE):
                        tp = psa.tile([128, 128], F32, tag="tp", bufs=2, space="PSUM")
                        nc.tensor.transpose(out=tp, in_=wt[:, ek * 128 : (ek + 1) * 128], identity=ident)
                        nc.vector.tensor_copy(out=wihT[:, ek, gi * 128 : (gi + 1) * 128].bitcast(F32R), in_=tp)
                # --- W_h0.T ---
                for hi in range(H // 128):
                    wt = sba.tile([128, E2], F32, tag="wld2", bufs=3)
                    nc.sync.dma_start(out=wt, in_=wh0[hi * 128 : (hi + 1) * 128, :])
                    for ek in range(K2E):
                        tp = psa.tile([128, 128], F32, tag="tp", bufs=2, space="PSUM")
                        nc.tensor.transpose(out=tp, in_=wt[:, ek * 128 : (ek + 1) * 128], identity=ident)
                        nc.vector.tensor_copy(out=wh0T[:, ek, hi * 128 : (hi + 1) * 128], in_=tp)
                # --- posemb.T ---
                for ek in range(KE):
                    tp = psa.tile([128, 128], F32, tag="tp", bufs=2, space="PSUM")
                    nc.tensor.transpose(out=tp, in_=posemb[:, ek * 128 : (ek + 1) * 128], identity=ident)
                    nc.vector.tensor_copy(out=posembT[:, ek * 128 : (ek + 1) * 128], in_=tp)

                # --- encoder gathers -> avgT (mean over TE) ---
                avgT = sba.tile([128, K2E, B], F32)  # (e2 within chunk, kc, b)
                avg_ps = psa.tile([128, KE, B], F32, tag="avg", space="PSUM")
                for b in range(B):
                    gat = sba.tile([TE, E], F32, tag="gatA", bufs=2)
                    nc.gpsimd.indirect_dma_start(
                        out=gat, out_offset=None, in_=ence,
                        in_offset=bass.IndirectOffsetOnAxis(ap=srcidx[:, b : b + 1], axis=0),
                    )
                    for ec in range(KE):
                        nc.tensor.matmul(
                            out=avg_ps[:, ec, b : b + 1],
                            lhsT=gat[:, ec * 128 : (ec + 1) * 128], rhs=ones_col,
                            start=True, stop=True,
                        )
                nc.vector.tensor_copy(out=avgT[:, 0:KE, :], in_=avg_ps)
                # positional half of the mean: constant across b
                pos_ps = psa.tile([128, KE], F32, tag="posps", space="PSUM")
                for ec in range(KE):
                    nc.tensor.matmul(
                        out=pos_ps[:, ec : ec + 1],
                        lhsT=posemb[:, ec * 128 : (ec + 1) * 128], rhs=ones_col,
                        start=True, stop=True,
                    )
                for ec in range(KE):
                    nc.vector.tensor_copy(
                        out=avgT[:, KE + ec, :],
                        in_=pos_ps[:, ec : ec + 1].to_broadcast([128, B]),
                    )

                # --- h0T = W_h0 @ avg.T (+ b_h0), then h0 ---
                h0_ps = psa.tile([128, KH * 32], F32, tag="h0ps", space="PSUM")
                for m in range(KH):
                    for k in range(K2E):
                        nc.tensor.matmul(
                            out=h0_ps[:, m * 32 : (m + 1) * 32],
                            lhsT=wh0T[:, k, m * 128 : (m + 1) * 128],
                            rhs=avgT[:, k, :],
                            start=(k == 0), stop=(k == K2E - 1),
                        )
                for m in range(KH):
                    nc.vector.tensor_scalar(
                        out=h0T[:, m * 32 : (m + 1) * 32].bitcast(F32R),
                        in0=h0_ps[:, m * 32 : (m + 1) * 32],
                        scalar1=bh0_sb[:, m : m + 1], scalar2=None, op0=OP.add,
                    )
                for m in range(KH):
                    tp2 = psa.tile([32, 128], F32, tag="tp", bufs=2, space="PSUM")
                    nc.tensor.transpose(out=tp2, in_=h0T[:, m * 32 : (m + 1) * 32], identity=ident)
                    nc.vector.tensor_copy(out=h0[:, m * 128 : (m + 1) * 128], in_=tp2)

                # --- Zx = X @ W_ih.T + bias, stored (t, b, 4H) ---
                for mt in range(B):  # token tile mt == sentence mt (rows t=0..127)
                    xg = sba.tile([TD, E], F32, tag="xg", bufs=2)
                    nc.gpsimd.indirect_dma_start(
                        out=xg, out_offset=None, in_=dece,
                        in_offset=bass.IndirectOffsetOnAxis(ap=tgtidx[:, mt : mt + 1], axis=0),
                    )
                    xT = sba.tile([128, KE * 128], F32, tag="xT", bufs=2)
                    for ek in range(KE):
                        tp = psa.tile([128, 128], F32, tag="tp", bufs=2, space="PSUM")
                        nc.tensor.transpose(out=tp, in_=xg[:, ek * 128 : (ek + 1) * 128], identity=ident)
                        nc.vector.tensor_copy(out=xT[:, ek * 128 : (ek + 1) * 128].bitcast(F32R), in_=tp)
                    for n in range(G // 512):
                        zps = psa.tile([128, 512], F32, tag="zx", bufs=2, space="PSUM")
                        for k in range(KE):
                            nc.tensor.matmul(
                                out=zps,
                                lhsT=r(xT[:, k * 128 : (k + 1) * 128]),
                                rhs=r(wihT[:, k, n * 512 : (n + 1) * 512]),
                                start=(k == 0), stop=False,
                            )
                        # bias broadcast via K=1 matmul: ones.T @ bias_row
                        nc.tensor.matmul(
                            out=zps, lhsT=ones_row,
                            rhs=bias_g[0:1, n * 512 : (n + 1) * 512],
                            start=False, stop=True,
                        )
                        zxo = sba.tile([TD, 512], F32, tag="zxo", bufs=3)
                        nc.vector.tensor_copy(out=zxo, in_=zps)
                        nc.sync.dma_start(out=zx_d[:, mt, n * 512 : (n + 1) * 512], in_=zxo)

            # ================= phase B: LSTM =================
            with (
                tc.tile_pool(name="wB", bufs=1) as wb,
                tc.tile_pool(name="psB", bufs=1, space="PSUM") as psb,
                tc.tile_pool(name="sbB", bufs=1) as sbb,
            ):
                whhT = wb.tile([128, KH, G], F32)    # (h, kc, gate)
                with tc.tile_pool(name="wldB", bufs=1) as wldb:
                    for gi in range(G // 128):
                        wt = wldb.tile([128, H], F32, tag="wld2", bufs=3)
                        nc.sync.dma_start(out=wt, in_=whh[gi * 128 : (gi + 1) * 128, :])
                        for hk in range(KH):
                            tp = psb.tile([128, 128], F32, tag="tp", bufs=2, space="PSUM")
                            nc.tensor.transpose(out=tp, in_=wt[:, hk * 128 : (hk + 1) * 128], identity=ident)
                            nc.vector.tensor_copy(out=whhT[:, hk, gi * 128 : (gi + 1) * 128].bitcast(F32R), in_=tp)

                c_prev = sbb.tile([B, H], F32, tag="c", bufs=2)
                nc.vector.tensor_copy(out=c_prev, in_=h0)
                hT_prev = h0T

                for t in range(TD):
                    zc = []
                    for n in range(G // 512):
                        zps = psb.tile([B, 512], F32, tag="zb", bufs=4, space="PSUM")
                        for k in range(KH):
                            nc.tensor.matmul(
                                out=zps,
                                lhsT=r(hT_prev[:, k * 32 : (k + 1) * 32]),
                                rhs=r(whhT[:, k, n * 512 : (n + 1) * 512]),
                                start=(k == 0), stop=(k == KH - 1),
                            )
                        zxt = sbb.tile([B, 512], F32, tag="zxt", bufs=8)
                        nc.sync.dma_start(out=zxt, in_=zx_d[t, :, n * 512 : (n + 1) * 512])
                        nc.vector.tensor_tensor(out=zxt, in0=zps, in1=zxt, op=OP.add)
                        zc.append(zxt)
                    # gate order in z: i | f | g | o, 512-wide chunks
                    si = sbb.tile([B, H], F32, tag="si", bufs=1)
                    tg = sbb.tile([B, H], F32, tag="tg", bufs=1)
                    sf = sbb.tile([B, H], F32, tag="sf", bufs=1)
                    for u in range(2):
                        nc.scalar.activation(out=si[:, u * 512 : (u + 1) * 512], in_=zc[0 + u], func=AF.Sigmoid)
                        nc.scalar.activation(out=sf[:, u * 512 : (u + 1) * 512], in_=zc[2 + u], func=AF.Sigmoid)
                        nc.scalar.activation(out=tg[:, u * 512 : (u + 1) * 512], in_=zc[4 + u], func=AF.Tanh)
                    nc.vector.tensor_mul(out=si, in0=si, in1=tg)      # i*g
                    nc.vector.tensor_mul(out=sf, in0=sf, in1=c_prev)  # f*c
                    c_new = sbb.tile([B, H], F32, tag="c", bufs=2)
                    nc.vector.tensor_add(out=c_new, in0=si, in1=sf)
                    nc.scalar.activation(out=si, in_=c_new, func=AF.Tanh)  # tanh(c)
                    for u in range(2):
                        nc.scalar.activation(out=tg[:, u * 512 : (u + 1) * 512], in_=zc[6 + u], func=AF.Sigmoid)
                    h_new = sbb.tile([B, H], F32, tag="h", bufs=2)
                    nc.vector.tensor_mul(out=h_new, in0=tg, in1=si)  # sigmoid(o)*tanh(c)
                    nc.sync.dma_start(out=lstm_d[:, t, :], in_=h_new)
                    hT_ps = psb.tile([128, KH * 32], F32, tag="hT", bufs=2, space="PSUM")
                    for m in range(KH):
                        nc.tensor.transpose(
                            out=hT_ps[:, m * 32 : (m + 1) * 32],
                            in_=h_new[:, m * 128 : (m + 1) * 128], identity=ident[:32, :32],
                        )
                    hT_new = sbb.tile([128, KH * 32], F32, tag="hTs", bufs=2)
                    nc.vector.tensor_copy(out=hT_new.bitcast(F32R), in_=hT_ps)
                    c_prev = c_new
                    hT_prev = hT_new

            # ============ phases C+D: attention + projection ============
            with (
                tc.tile_pool(name="wC", bufs=1) as wc,
                tc.tile_pool(name="psC", bufs=1, space="PSUM") as psc,
                tc.tile_pool(name="sbC", bufs=1) as sbc,
            ):
                # --- phase C: per-sentence attention -> combT blocks ---
                for b in range(B):
                    gat = sbc.tile([TE, E], F32, tag="gatC", bufs=2)
                    nc.gpsimd.indirect_dma_start(
                        out=gat, out_offset=None, in_=ence,
                        in_offset=bass.IndirectOffsetOnAxis(ap=srcidx[:, b : b + 1], axis=0),
                    )
                    kbT = sbc.tile([128, KE * 128], F32, tag="kbT", bufs=2)
                    for ek in range(KE):
                        tp = psc.tile([128, 128], F32, tag="tp", bufs=2, space="PSUM")
                        nc.tensor.transpose(out=tp, in_=gat[:, ek * 128 : (ek + 1) * 128], identity=ident)
                        nc.vector.tensor_copy(out=kbT[:, ek * 128 : (ek + 1) * 128], in_=tp)

                    lstm_b = sbc.tile([TD, H], F32, tag="lstmb", bufs=2)
                    nc.sync.dma_start(out=lstm_b, in_=lstm_d[b])
                    combT = sbc.tile([128, 16, TD], F32, tag="combT", bufs=2)
                    for m in range(KH):
                        tp = psc.tile([128, 128], F32, tag="tp", bufs=2, space="PSUM")
                        nc.tensor.transpose(out=tp, in_=lstm_b[:, m * 128 : (m + 1) * 128], identity=ident)
                        nc.vector.tensor_copy(out=combT[:, m, :].bitcast(F32R), in_=tp)
                    # queries: h_{t-1}; col 0 is h0
                    qT = sbc.tile([128, KH, TD], F32, tag="qT", bufs=2)
                    for k in range(KH):
                        nc.vector.tensor_copy(out=qT[:, k, 1:TD], in_=combT[:, k, 0 : TD - 1])
                        nc.vector.tensor_copy(out=qT[:, k, 0:1], in_=h0T[:, k * 32 + b : k * 32 + b + 1])

                    mrow = sbc.tile([1, TE], F32, tag="mask", bufs=2)
                    nc.vector.tensor_scalar(
                        out=mrow, in0=iota_f,
                        scalar1=slen_f[0:1, b : b + 1], scalar2=-30.0,
                        op0=OP.is_ge, op1=OP.mult,
                    )
                    e_ps = psc.tile([TD, TE], F32, tag="e", space="PSUM")
                    for k in range(KH):
                        rhs = kbT[:, k * 128 : (k + 1) * 128] if k < KE else posembT[:, (k - KE) * 128 : (k - KE + 1) * 128]
                        nc.tensor.matmul(out=e_ps, lhsT=qT[:, k, :], rhs=rhs,
                                         start=(k == 0), stop=False)
                    nc.tensor.matmul(out=e_ps, lhsT=ones_row, rhs=mrow,
                                     start=False, stop=True)
                    p_sb = sbc.tile([TD, TE], F32, tag="p", bufs=2)
                    s_sb = sbc.tile([TD, 1], F32, tag="s", bufs=2)
                    nc.scalar.activation(out=p_sb, in_=e_ps, func=AF.Exp, accum_out=s_sb)
                    rs = sbc.tile([TD, 1], F32, tag="rs", bufs=2)
                    nc.vector.reciprocal(out=rs, in_=s_sb)
                    attn = sbc.tile([TD, TE], F32, tag="attn", bufs=2)
                    nc.vector.tensor_scalar_mul(out=attn, in0=p_sb, scalar1=rs)
                    atp = psc.tile([TE, TD], F32, tag="tp", bufs=2, space="PSUM")
                    nc.tensor.transpose(out=atp, in_=attn, identity=ident)
                    attnT = sbc.tile([TE, TD], F32, tag="attnT", bufs=2)
                    nc.vector.tensor_copy(out=attnT, in_=atp)
                    ct_ps = psc.tile([128, KH * 128], F32, tag="ct", space="PSUM")
                    for m in range(KH):
                        lhs = gat[:, m * 128 : (m + 1) * 128] if m < KE else posemb[:, (m - KE) * 128 : (m - KE + 1) * 128]
                        nc.tensor.matmul(out=ct_ps[:, m * 128 : (m + 1) * 128],
                                         lhsT=lhs, rhs=attnT, start=True, stop=True)
                    for m in range(KH):
                        nc.vector.tensor_copy(out=combT[:, KH + m, :].bitcast(F32R), in_=ct_ps[:, m * 128 : (m + 1) * 128])
                    nc.sync.dma_start(out=combT_d[b].rearrange("k p t -> p k t"), in_=combT)

                # --- phase D: vocab-sharded projection in 4 quarters ---
                wqT = wc.tile([128, 16, QW], F32)    # (feat, kc, vocab-in-quarter)
                for q in range(NQ):
                    bq = sbc.tile([1, QW], F32, tag="bq", bufs=2)
                    nc.sync.dma_start(out=bq, in_=bout[None, q * QW : (q + 1) * QW])
                    for vc in range(QW // 128):
                        wt = sbc.tile([128, E2 + H], F32, tag="wld3", bufs=2)
                        nc.sync.dma_start(out=wt, in_=wout[q * QW + vc * 128 : q * QW + (vc + 1) * 128, :])
                        for k in range(16):
                            tp = psc.tile([128, 128], F32, tag="tp", bufs=2, space="PSUM")
                            nc.tensor.transpose(out=tp, in_=wt[:, k * 128 : (k + 1) * 128], identity=ident)
                            nc.vector.tensor_copy(out=wqT[:, k, vc * 128 : (vc + 1) * 128].bitcast(F32R), in_=tp)
                    for mt in range(B):
                        cT = sbc.tile([128, 16, TD], F32, tag="cT", bufs=2)
                        nc.sync.dma_start(out=cT, in_=combT_d[mt].rearrange("k p t -> p k t"))
                        o_sb = sbc.tile([128, QW], F32, tag="osb", bufs=3)
                        for nb in range(QW // 512):
                            po = psc.tile([128, 512], F32, tag="po", bufs=3, space="PSUM")
                            for k in range(16):
                                nc.tensor.matmul(
                                    out=po,
                                    lhsT=r(cT[:, k, :]),
                                    rhs=r(wqT[:, k, nb * 512 : (nb + 1) * 512]),
                                    start=(k == 0), stop=False,
                                )
                            nc.tensor.matmul(
                                out=po, lhsT=ones_row,
                                rhs=bq[0:1, nb * 512 : (nb + 1) * 512],
                                start=False, stop=True,
                            )
                            nc.vector.tensor_copy(out=o_sb[:, nb * 512 : (nb + 1) * 512], in_=po)
                        nc.sync.dma_start(
                            out=out[mt * 128 : (mt + 1) * 128, q * QW : (q + 1) * QW],
                            in_=o_sb,
                        )
    return nc


_NC_CACHE = None


def _get_nc():
    global _NC_CACHE
    if _NC_CACHE is None:
        nc = build_nc()
        if not nc.is_finalized():
            nc.finalize()  # Bacc passes: wait-splitting, reg alloc, act tables
        _NC_CACHE = nc
    return _NC_CACHE


def _in_maps(inputs):
    f32 = lambda x: np.ascontiguousarray(np.asarray(x, dtype=np.float32))
    i32 = lambda x: np.ascontiguousarray(np.asarray(x, dtype=np.int32))
    common = {
        "tgt": i32(inputs["target_sentences"]),
        "srcs": i32(inputs["source_sentences"]),
        "slen": i32(inputs["source_lengths"]),
        "arange": np.arange(TE, dtype=np.int32),
        "ence": f32(inputs["enc_emb"]),
        "dece": f32(inputs["dec_emb"]),
        "pose": f32(np.asarray(inputs["pos_emb"])[:TE]),
        "wih": f32(inputs["W_ih"]),
        "whh": f32(inputs["W_hh"]),
        "bihh": f32(np.asarray(inputs["b_ih"], np.float32) + np.asarray(inputs["b_hh"], np.float32)),
        "wh0": f32(inputs["W_h0"]),
        "bh0": f32(inputs["b_h0"]),
    }
    wout = f32(inputs["W_out"])
    bout = f32(inputs["b_out"])
    wout_pad = np.concatenate([wout, np.zeros((VPAD - V, E2 + H), np.float32)], axis=0)
    bout_pad = np.concatenate([bout, np.zeros(VPAD - V, np.float32)])
    maps = []
    for c in range(NCORES):
        m = dict(common)
        m["wout"] = np.ascontiguousarray(wout_pad[c * VS : (c + 1) * VS])
        m["bout"] = np.ascontiguousarray(bout_pad[c * VS : (c + 1) * VS])
        maps.append(m)
    return maps


def run(inputs, trace=False, **kwargs):
    """Run on 8 cores; returns (output (B, TD, V) fp32, BassKernelResults)."""
    nc = _get_nc()
    res = run_bass_kernel_spmd(
        nc, _in_maps(inputs), core_ids=list(range(NCORES)), trace=trace, **kwargs
    )
    shards = [res.results[c]["out"] for c in range(NCORES)]
    full = np.concatenate(shards, axis=1)[:, :V]
    return full.reshape(B, TD, V).astype(np.float32, copy=False), res


def kernel(**inputs) -> np.ndarray:
    out, _ = run(inputs, trace=False)
    return out


# revision 24
# speedup vs baseline: 1.0017x; 1.0017x over previous
"""Trainium2 Bass kernel for nn_Decoder (seq2seq LSTM decoder w/ attention).

Strategy (8 NeuronCores, SPMD, zero collectives):
  - The LSTM recurrence, embedding gathers and attention are replicated on
    every core (the recurrence is weight-stream bound on the PE, so running
    the full batch B=32 costs the same as any shard of it; per-step
    collectives would cost ~20us each and are a non-starter).
  - The 32000-wide output projection is sharded over vocab: each core owns a
    4096-row slice of W_out (32000 padded to 32768 = 8*4096) and produces
    logits[:, shard]. The host concatenates shards and drops the padding.

Per-core pipeline:
  A: weight transposes (PE), encoder-embedding gathers, avg/h0, and the
     time-parallel LSTM input projection Zx = X @ W_ih.T + b (to DRAM).
  B: 128 sequential LSTM steps; z = Zx[t] + h @ W_hh.T via PE matmuls with
     the hidden state transposed into lhsT form each step by PE transposes.
  C: per-sentence attention (masked softmax via an additive -30 mask row
     folded into the energy matmul accumulation) producing combined.T
     blocks in DRAM.
  D: the projection combined @ W_out_shard.T + b_out in 4 vocab quarters,
     keeping each transposed weight quarter resident in SBUF.

All large matmuls run as float32r (full PE rate); data stays fp32 end to end.
"""

import numpy as np

import concourse.bass as bass
import concourse.tile as tile
from concourse import bacc
from concourse import mybir
from concourse.bass_utils import run_bass_kernel_spmd
from concourse.masks import make_identity

B, TD, TE = 32, 128, 128
E, H = 512, 1024
E2 = 2 * E                 # 1024
G = 4 * H                  # 4096 (gates)
KE, KH, K2E = E // 128, H // 128, E2 // 128   # 4, 8, 8
V = 32000
VPAD = 32768               # 8 * 4096
VS = VPAD // 8             # 4096 per-core vocab shard (padded)
NQ = 4                     # vocab quarters in phase D
QW = VS // NQ              # 1024 cols per quarter
NCORES = 8

F32 = mybir.dt.float32
F32R = mybir.dt.float32r
I32 = mybir.dt.int32
AF = mybir.ActivationFunctionType
OP = mybir.AluOpType


def r(ap):
    """View an fp32 AP as float32r for full-rate PE matmuls."""
    return ap.bitcast(F32R)


def build_nc():
    nc = bacc.Bacc()

    tgt = nc.dram_tensor("tgt", [B, TD], I32, kind="ExternalInput").ap()
    srcs = nc.dram_tensor("srcs", [B, TE], I32, kind="ExternalInput").ap()
    slen = nc.dram_tensor("slen", [B], I32, kind="ExternalInput").ap()
    arange = nc.dram_tensor("arange", [TE], I32, kind="ExternalInput").ap()
    ence = nc.dram_tensor("ence", [V, E], F32, kind="ExternalInput").ap()
    dece = nc.dram_tensor("dece", [V, E], F32, kind="ExternalInput").ap()
    pose = nc.dram_tensor("pose", [TE, E], F32, kind="ExternalInput").ap()
    wih = nc.dram_tensor("wih", [G, E], F32, kind="ExternalInput").ap()
    whh = nc.dram_tensor("whh", [G, H], F32, kind="ExternalInput").ap()
    bihh = nc.dram_tensor("bihh", [G], F32, kind="ExternalInput").ap()
    wh0 = nc.dram_tensor("wh0", [H, E2], F32, kind="ExternalInput").ap()
    bh0 = nc.dram_tensor("bh0", [H], F32, kind="ExternalInput").ap()
    wout = nc.dram_tensor("wout", [VS, E2 + H], F32, kind="ExternalInput").ap()
    bout = nc.dram_tensor("bout", [VS], F32, kind="ExternalInput").ap()
    out = nc.dram_tensor("out", [B * TD, VS], F32, kind="ExternalOutput").ap()

    with tile.TileContext(nc) as tc:
        with (
            tc.tile_pool(name="dram", bufs=1, space="DRAM") as dp,
            tc.tile_pool(name="const", bufs=1) as cp,
        ):
            zx_d = dp.tile([TD, B, G], F32)          # LSTM input proj, (t, b, 4H)
            lstm_d = dp.tile([B, TD, H], F32)        # h_t rows
            combT_d = dp.tile([B, 16, 128, TD], F32) # combined.T blocks per sentence

            # ---------------- constants (whole-kernel residents) ----------------
            ident = cp.tile([128, 128], F32)
            make_identity(nc, ident)
            ones_col = cp.tile([128, 1], F32)        # 1/TE for the encoder mean
            nc.vector.memset(ones_col, 1.0 / TE)
            ones_row = cp.tile([1, 128], F32)        # lhsT for mask broadcast matmul
            nc.vector.memset(ones_row, 1.0)

            posemb = cp.tile([TE, E], F32)
            nc.sync.dma_start(out=posemb, in_=pose)
            bh0_sb = cp.tile([128, KH], F32)
            nc.sync.dma_start(out=bh0_sb, in_=bh0.rearrange("(k p) -> p k", p=128))
            srcidx = cp.tile([TE, B], I32)
            nc.sync.dma_start(out=srcidx, in_=srcs.rearrange("b t -> t b"))
            tgtidx = cp.tile([TD, B], I32)
            nc.sync.dma_start(out=tgtidx, in_=tgt.rearrange("b t -> t b"))
            iota_i = cp.tile([1, TE], I32)
            nc.sync.dma_start(out=iota_i, in_=arange[None, :])
            iota_f = cp.tile([1, TE], F32)
            nc.vector.tensor_copy(out=iota_f, in_=iota_i)
            slen_i = cp.tile([1, B], I32)
            nc.sync.dma_start(out=slen_i, in_=slen[None, :])
            slen_f = cp.tile([1, B], F32)
            nc.vector.tensor_copy(out=slen_f, in_=slen_i)

            posembT = cp.tile([128, KE * 128], F32)  # (e within chunk, ec*128 + t)
            h0T = cp.tile([128, KH * 32], F32)       # (h within chunk, k*32 + b)
            h0 = cp.tile([B, H], F32)

            # ================= phase A =================
            with (
                tc.tile_pool(name="wA", bufs=1) as wa,
                tc.tile_pool(name="psA", bufs=1, space="PSUM") as psa,
                tc.tile_pool(name="sbA", bufs=1) as sba,
            ):
                wihT = wa.tile([128, KE, G], F32)    # (e, ec, gate)
                wh0T = wa.tile([128, K2E, H], F32)   # (e2, kc, h)
                bias_g = sba.tile([1, G], F32)
                nc.sync.dma_start(out=bias_g, in_=bihh[None, :])

                # --- W_ih.T ---
                for gi in range(G // 128):
                    wt = sba.tile([128, E], F32, tag="wld", bufs=3)
                    nc.sync.dma_start(out=wt, in_=wih[gi * 128 : (gi + 1) * 128, :])
                    for ek in range(KE):
                        tp = psa.tile([128, 128], F32, tag="tp", bufs=4, space="PSUM")
                        nc.tensor.transpose(out=tp, in_=wt[:, ek * 128 : (ek + 1) * 128], identity=ident)
                        nc.vector.tensor_copy(out=wihT[:, ek, gi * 128 : (gi + 1) * 128].bitcast(F32R), in_=tp)
                # --- W_h0.T ---
                for hi in range(H // 128):
                    wt = sba.tile([128, E2], F32, tag="wld2", bufs=3)
                    nc.sync.dma_start(out=wt, in_=wh0[hi * 128 : (hi + 1) * 128, :])
                    for ek in range(K2E):
                        tp = psa.tile([128, 128], F32, tag="tp", bufs=4, space="PSUM")
                        nc.tensor.transpose(out=tp, in_=wt[:, ek * 128 : (ek + 1) * 128], identity=ident)
                        nc.vector.tensor_copy(out=wh0T[:, ek, hi * 128 : (hi + 1) * 128], in_=tp)
                # --- posemb.T ---
                for ek in range(KE):
                    tp = psa.tile([128, 128], F32, tag="tp", bufs=4, space="PSUM")
                    nc.tensor.transpose(out=tp, in_=posemb[:, ek * 128 : (ek + 1) * 128], identity=ident)
                    nc.vector.tensor_copy(out=posembT[:, ek * 128 : (ek + 1) * 128], in_=tp)

                # --- encoder gathers -> avgT (mean over TE) ---
                avgT = sba.tile([128, K2E, B], F32)  # (e2 within chunk, kc, b)
                avg_ps = psa.tile([128, KE, B], F32, tag="avg", space="PSUM")
                for b in range(B):
                    gat = sba.tile([TE, E], F32, tag="gatA", bufs=2)
                    nc.gpsimd.indirect_dma_start(
                        out=gat, out_offset=None, in_=ence,
                        in_offset=bass.IndirectOffsetOnAxis(ap=srcidx[:, b : b + 1], axis=0),
                    )
                    for ec in range(KE):
                        nc.tensor.matmul(
                            out=avg_ps[:, ec, b : b + 1],
                            lhsT=gat[:, ec * 128 : (ec + 1) * 128], rhs=ones_col,
                            start=True, stop=True,
                        )
                nc.vector.tensor_copy(out=avgT[:, 0:KE, :], in_=avg_ps)
                # positional half of the mean: constant across b
                pos_ps = psa.tile([128, KE], F32, tag="h0ps", space="PSUM")
                for ec in range(KE):
                    nc.tensor.matmul(
                        out=pos_ps[:, ec : ec + 1],
                        lhsT=posemb[:, ec * 128 : (ec + 1) * 128], rhs=ones_col,
                        start=True, stop=True,
                    )
                for ec in range(KE):
                    nc.vector.tensor_copy(
                        out=avgT[:, KE + ec, :],
                        in_=pos_ps[:, ec : ec + 1].to_broadcast([128, B]),
                    )

                # --- h0T = W_h0 @ avg.T (+ b_h0), then h0 ---
                h0_ps = psa.tile([128, KH * 32], F32, tag="h0ps", space="PSUM")
                for m in range(KH):
                    for k in range(K2E):
                        nc.tensor.matmul(
                            out=h0_ps[:, m * 32 : (m + 1) * 32],
                            lhsT=wh0T[:, k, m * 128 : (m + 1) * 128],
                            rhs=avgT[:, k, :],
                            start=(k == 0), stop=(k == K2E - 1),
                        )
                for m in range(KH):
                    nc.vector.tensor_scalar(
                        out=h0T[:, m * 32 : (m + 1) * 32].bitcast(F32R),
                        in0=h0_ps[:, m * 32 : (m + 1) * 32],
                        scalar1=bh0_sb[:, m : m + 1], scalar2=None, op0=OP.add,
                    )
                for m in range(KH):
                    tp2 = psa.tile([32, 128], F32, tag="tp", bufs=4, space="PSUM")
                    nc.tensor.transpose(out=tp2, in_=h0T[:, m * 32 : (m + 1) * 32], identity=ident)
                    nc.vector.tensor_copy(out=h0[:, m * 128 : (m + 1) * 128], in_=tp2)

                # --- Zx = X @ W_ih.T + bias, stored (t, b, 4H) ---
                for mt in range(B):  # token tile mt == sentence mt (rows t=0..127)
                    xg = sba.tile([TD, E], F32, tag="xg", bufs=2)
                    nc.gpsimd.indirect_dma_start(
                        out=xg, out_offset=None, in_=dece,
                        in_offset=bass.IndirectOffsetOnAxis(ap=tgtidx[:, mt : mt + 1], axis=0),
                    )
                    xT = sba.tile([128, KE * 128], F32, tag="xT", bufs=2)
                    for ek in range(KE):
                        tp = psa.tile([128, 128], F32, tag="tp", bufs=4, space="PSUM")
                        nc.tensor.transpose(out=tp, in_=xg[:, ek * 128 : (ek + 1) * 128], identity=ident)
                        nc.vector.tensor_copy(out=xT[:, ek * 128 : (ek + 1) * 128].bitcast(F32R), in_=tp)
                    for n in range(G // 512):
                        zps = psa.tile([128, 512], F32, tag="zx", bufs=2, space="PSUM")
                        for k in range(KE):
                            nc.tensor.matmul(
                                out=zps,
                                lhsT=r(xT[:, k * 128 : (k + 1) * 128]),
                                rhs=r(wihT[:, k, n * 512 : (n + 1) * 512]),
                                start=(k == 0), stop=False,
                            )
                        # bias broadcast via K=1 matmul: ones.T @ bias_row
                        nc.tensor.matmul(
                            out=zps, lhsT=ones_row,
                            rhs=bias_g[0:1, n * 512 : (n + 1) * 512],
                            start=False, stop=True,
                        )
                        zxo = sba.tile([TD, 512], F32, tag="zxo", bufs=3)
                        nc.vector.tensor_copy(out=zxo, in_=zps)
                        nc.sync.dma_start(out=zx_d[:, mt, n * 512 : (n + 1) * 512], in_=zxo)

            # ================= phase B: LSTM =================
            with (
                tc.tile_pool(name="wB", bufs=1) as wb,
                tc.tile_pool(name="psB", bufs=1, space="PSUM") as psb,
                tc.tile_pool(name="sbB", bufs=1) as sbb,
            ):
                whhT = wb.tile([128, KH, G], F32)    # (h, kc, gate)
                with tc.tile_pool(name="wldB", bufs=1) as wldb:
                    for gi in range(G // 128):
                        wt = wldb.tile([128, H], F32, tag="wld2", bufs=3)
                        nc.sync.dma_start(out=wt, in_=whh[gi * 128 : (gi + 1) * 128, :])
                        for hk in range(KH):
                            tp = psb.tile([128, 128], F32, tag="zb", bufs=4, space="PSUM")
                            nc.tensor.transpose(out=tp, in_=wt[:, hk * 128 : (hk + 1) * 128], identity=ident)
                            nc.vector.tensor_copy(out=whhT[:, hk, gi * 128 : (gi + 1) * 128].bitcast(F32R), in_=tp)

                c_prev = sbb.tile([B, H], F32, tag="c", bufs=2)
                nc.vector.tensor_copy(out=c_prev, in_=h0)
                hT_prev = h0T

                for t in range(TD):
                    zc = []
                    for n in range(G // 512):
                        zps = psb.tile([B, 512], F32, tag="zb", bufs=4, space="PSUM")
                        for k in range(KH):
                            nc.tensor.matmul(
                                out=zps,
                                lhsT=r(hT_prev[:, k * 32 : (k + 1) * 32]),
                                rhs=r(whhT[:, k, n * 512 : (n + 1) * 512]),
                                start=(k == 0), stop=(k == KH - 1),
                            )
                        zxt = sbb.tile([B, 512], F32, tag="zxt", bufs=8)
                        nc.sync.dma_start(out=zxt, in_=zx_d[t, :, n * 512 : (n + 1) * 512])
                        nc.vector.tensor_tensor(out=zxt, in0=zps, in1=zxt, op=OP.add)
                        zc.append(zxt)
                    # gate order in z: i | f | g | o, 512-wide chunks
                    si = sbb.tile([B, H], F32, tag="si", bufs=1)
                    tg = sbb.tile([B, H], F32, tag="tg", bufs=1)
                    sf = sbb.tile([B, H], F32, tag="sf", bufs=1)
                    for u in range(2):
                        nc.scalar.activation(out=si[:, u * 512 : (u + 1) * 512], in_=zc[0 + u], func=AF.Sigmoid)
                        nc.scalar.activation(out=sf[:, u * 512 : (u + 1) * 512], in_=zc[2 + u], func=AF.Sigmoid)
                        nc.scalar.activation(out=tg[:, u * 512 : (u + 1) * 512], in_=zc[4 + u], func=AF.Tanh)
                    nc.vector.tensor_mul(out=si, in0=si, in1=tg)      # i*g
                    nc.vector.tensor_mul(out=sf, in0=sf, in1=c_prev)  # f*c
                    c_new = sbb.tile([B, H], F32, tag="c", bufs=2)
                    nc.vector.tensor_add(out=c_new, in0=si, in1=sf)
                    nc.scalar.activation(out=si, in_=c_new, func=AF.Tanh)  # tanh(c)
                    for u in range(2):
                        nc.scalar.activation(out=tg[:, u * 512 : (u + 1) * 512], in_=zc[6 + u], func=AF.Sigmoid)
                    h_new = sbb.tile([B, H], F32, tag="h", bufs=2)
                    nc.vector.tensor_mul(out=h_new, in0=tg, in1=si)  # sigmoid(o)*tanh(c)
                    nc.sync.dma_start(out=lstm_d[:, t, :], in_=h_new)
                    hT_ps = psb.tile([128, KH * 32], F32, tag="hT", bufs=2, space="PSUM")
                    for m in range(KH):
                        nc.tensor.transpose(
                            out=hT_ps[:, m * 32 : (m + 1) * 32],
                            in_=h_new[:, m * 128 : (m + 1) * 128], identity=ident[:32, :32],
                        )
                    hT_new = sbb.tile([128, KH * 32], F32, tag="hTs", bufs=2)
                    nc.vector.tensor_copy(out=hT_new.bitcast(F32R), in_=hT_ps)
                    c_prev = c_new
                    hT_prev = hT_new

            # ============ phases C+D: attention + projection ============
            with (
                tc.tile_pool(name="wC", bufs=1) as wc,
                tc.tile_pool(name="psC", bufs=1, space="PSUM") as psc,
                tc.tile_pool(name="sbC", bufs=1) as sbc,
            ):
                # --- phase C: per-sentence attention -> combT blocks ---
                for b in range(B):
                    gat = sbc.tile([TE, E], F32, tag="gatC", bufs=2)
                    nc.gpsimd.indirect_dma_start(
                        out=gat, out_offset=None, in_=ence,
                        in_offset=bass.IndirectOffsetOnAxis(ap=srcidx[:, b : b + 1], axis=0),
                    )
                    kbT = sbc.tile([128, KE * 128], F32, tag="kbT", bufs=2)
                    for ek in range(KE):
                        tp = psc.tile([128, 128], F32, tag="tp", bufs=2, space="PSUM")
                        nc.tensor.transpose(out=tp, in_=gat[:, ek * 128 : (ek + 1) * 128], identity=ident)
                        nc.vector.tensor_copy(out=kbT[:, ek * 128 : (ek + 1) * 128], in_=tp)

                    lstm_b = sbc.tile([TD, H], F32, tag="lstmb", bufs=2)
                    nc.sync.dma_start(out=lstm_b, in_=lstm_d[b])
                    combT = sbc.tile([128, 16, TD], F32, tag="combT", bufs=2)
                    for m in range(KH):
                        tp = psc.tile([128, 128], F32, tag="tp", bufs=2, space="PSUM")
                        nc.tensor.transpose(out=tp, in_=lstm_b[:, m * 128 : (m + 1) * 128], identity=ident)
                        nc.vector.tensor_copy(out=combT[:, m, :].bitcast(F32R), in_=tp)
                    # queries: h_{t-1}; col 0 is h0
                    qT = sbc.tile([128, KH, TD], F32, tag="qT", bufs=2)
                    for k in range(KH):
                        nc.vector.tensor_copy(out=qT[:, k, 1:TD], in_=combT[:, k, 0 : TD - 1])
                        nc.vector.tensor_copy(out=qT[:, k, 0:1], in_=h0T[:, k * 32 + b : k * 32 + b + 1])

                    mrow = sbc.tile([1, TE], F32, tag="mask", bufs=2)
                    nc.vector.tensor_scalar(
                        out=mrow, in0=iota_f,
                        scalar1=slen_f[0:1, b : b + 1], scalar2=-30.0,
                        op0=OP.is_ge, op1=OP.mult,
                    )
                    e_ps = psc.tile([TD, TE], F32, tag="e", space="PSUM")
                    for k in range(KH):
                        rhs = kbT[:, k * 128 : (k + 1) * 128] if k < KE else posembT[:, (k - KE) * 128 : (k - KE + 1) * 128]
                        nc.tensor.matmul(out=e_ps, lhsT=qT[:, k, :], rhs=rhs,
                                         start=(k == 0), stop=False)
                    nc.tensor.matmul(out=e_ps, lhsT=ones_row, rhs=mrow,
                                     start=False, stop=True)
                    p_sb = sbc.tile([TD, TE], F32, tag="p", bufs=2)
                    s_sb = sbc.tile([TD, 1], F32, tag="s", bufs=2)
                    nc.scalar.activation(out=p_sb, in_=e_ps, func=AF.Exp, accum_out=s_sb)
                    rs = sbc.tile([TD, 1], F32, tag="rs", bufs=2)
                    nc.vector.reciprocal(out=rs, in_=s_sb)
                    attn = sbc.tile([TD, TE], F32, tag="attn", bufs=2)
                    nc.vector.tensor_scalar_mul(out=attn, in0=p_sb, scalar1=rs)
                    atp = psc.tile([TE, TD], F32, tag="tp", bufs=2, space="PSUM")
                    nc.tensor.transpose(out=atp, in_=attn, identity=ident)
                    attnT = sbc.tile([TE, TD], F32, tag="attnT", bufs=2)
                    nc.vector.tensor_copy(out=attnT, in_=atp)
                    ct_ps = psc.tile([128, KH * 128], F32, tag="ct", space="PSUM")
                    for m in range(KH):
                        lhs = gat[:, m * 128 : (m + 1) * 128] if m < KE else posemb[:, (m - KE) * 128 : (m - KE + 1) * 128]
                        nc.tensor.matmul(out=ct_ps[:, m * 128 : (m + 1) * 128],
                                         lhsT=lhs, rhs=attnT, start=True, stop=True)
                    for m in range(KH):
                        nc.vector.tensor_copy(out=combT[:, KH + m, :].bitcast(F32R), in_=ct_ps[:, m * 128 : (m + 1) * 128])
                    nc.sync.dma_start(out=combT_d[b].rearrange("k p t -> p k t"), in_=combT)

                # --- phase D: vocab-sharded projection in 4 quarters ---
                wqT = wc.tile([128, 16, QW], F32)    # (feat, kc, vocab-in-quarter)
                for q in range(NQ):
                    bq = sbc.tile([1, QW], F32, tag="bq", bufs=2)
                    nc.sync.dma_start(out=bq, in_=bout[None, q * QW : (q + 1) * QW])
                    for vc in range(QW // 128):
                        wt = sbc.tile([128, E2 + H], F32, tag="wld3", bufs=2)
                        nc.sync.dma_start(out=wt, in_=wout[q * QW + vc * 128 : q * QW + (vc + 1) * 128, :])
                        for k in range(16):
                            tp = psc.tile([128, 128], F32, tag="tp", bufs=2, space="PSUM")
                            nc.tensor.transpose(out=tp, in_=wt[:, k * 128 : (k + 1) * 128], identity=ident)
                            nc.vector.tensor_copy(out=wqT[:, k, vc * 128 : (vc + 1) * 128].bitcast(F32R), in_=tp)
                    for mt in range(B):
                        cT = sbc.tile([128, 16, TD], F32, tag="cT", bufs=3)
                        nc.sync.dma_start(out=cT, in_=combT_d[mt].rearrange("k p t -> p k t"))
                        o_sb = sbc.tile([128, QW], F32, tag="osb", bufs=3)
                        for nb in range(QW // 512):
                            po = psc.tile([128, 512], F32, tag="po", bufs=3, space="PSUM")
                            for k in range(16):
                                nc.tensor.matmul(
                                    out=po,
                                    lhsT=r(cT[:, k, :]),
                                    rhs=r(wqT[:, k, nb * 512 : (nb + 1) * 512]),
                                    start=(k == 0), stop=False,
                                )
                            nc.tensor.matmul(
                                out=po, lhsT=ones_row,
                                rhs=bq[0:1, nb * 512 : (nb + 1) * 512],
                                start=False, stop=True,
                            )
                            nc.vector.tensor_copy(out=o_sb[:, nb * 512 : (nb + 1) * 512], in_=po)
                        nc.sync.dma_start(
                            out=out[mt * 128 : (mt + 1) * 128, q * QW : (q + 1) * QW],
                            in_=o_sb,
                        )
    return nc


_NC_CACHE = None


def _get_nc():
    global _NC_CACHE
    if _NC_CACHE is None:
        nc = build_nc()
        if not nc.is_finalized():
            nc.finalize()  # Bacc passes: wait-splitting, reg alloc, act tables
        _NC_CACHE = nc
    return _NC_CACHE


def _in_maps(inputs):
    f32 = lambda x: np.ascontiguousarray(np.asarray(x, dtype=np.float32))
    i32 = lambda x: np.ascontiguousarray(np.asarray(x, dtype=np.int32))
    common = {
        "tgt": i32(inputs["target_sentences"]),
        "srcs": i32(inputs["source_sentences"]),
        "slen": i32(inputs["source_lengths"]),
        "arange": np.arange(TE, dtype=np.int32),
        "ence": f32(inputs["enc_emb"]),
        "dece": f32(inputs["dec_emb"]),
        "pose": f32(np.asarray(inputs["pos_emb"])[:TE]),
        "wih": f32(inputs["W_ih"]),
        "whh": f32(inputs["W_hh"]),
        "bihh": f32(np.asarray(inputs["b_ih"], np.float32) + np.asarray(inputs["b_hh"], np.float32)),
        "wh0": f32(inputs["W_h0"]),
        "bh0": f32(inputs["b_h0"]),
    }
    wout = f32(inputs["W_out"])
    bout = f32(inputs["b_out"])
    wout_pad = np.concatenate([wout, np.zeros((VPAD - V, E2 + H), np.float32)], axis=0)
    bout_pad = np.concatenate([bout, np.zeros(VPAD - V, np.float32)])
    maps = []
    for c in range(NCORES):
        m = dict(common)
        m["wout"] = np.ascontiguousarray(wout_pad[c * VS : (c + 1) * VS])
        m["bout"] = np.ascontiguousarray(bout_pad[c * VS : (c + 1) * VS])
        maps.append(m)
    return maps


def run(inputs, trace=False, **kwargs):
    """Run on 8 cores; returns (output (B, TD, V) fp32, BassKernelResults)."""
    nc = _get_nc()
    res = run_bass_kernel_spmd(
        nc, _in_maps(inputs), core_ids=list(range(NCORES)), trace=trace, **kwargs
    )
    shards = [res.results[c]["out"] for c in range(NCORES)]
    full = np.concatenate(shards, axis=1)[:, :V]
    return full.reshape(B, TD, V).astype(np.float32, copy=False), res


def kernel(**inputs) -> np.ndarray:
    out, _ = run(inputs, trace=False)
    return out
